# revision 12
# baseline (speedup 1.0000x reference)
"""KNN-interpolate (PyG knn_interpolate style) Bass kernel for Trainium2.

Problem: for each of 16384 fine points (pos_h), find the K=3 nearest of
4096 coarse points (pos_l), gather their features (x, 256-dim), and
inverse-distance-weight them into the output.

Strategy (data-parallel over pos_h, 8 cores, 2048 queries each):
  - PE: score matrix t[i,j] = 2*ph_i . pl_j - |pl_j|^2 via a contract-dim-4
    matmul (the -|pl|^2 term rides along as a 4th "coordinate" against a
    constant-1 row in the query matrix).  argmax_j t = argmin_j d2.
  - DVE: max8 + max_index per 128-query tile -> top-3 scores + indices.
  - GPSIMD indirect DMA: gather the 3 neighbor feature rows per query.
  - DVE: w = 1/clip(d2, eps) with d2 = |ph|^2 - t, weighted sum, normalize.
"""

import os
import sys
from contextlib import ExitStack

import numpy as np

sys.path.insert(0, "/opt/trn_rl_repo")

import concourse.bacc as bacc
import concourse.bass as bass
import concourse.mybir as mybir
import concourse.tile as tile
from concourse import bass_utils, masks
from concourse.bass import IndirectOffsetOnAxis

N_CORES = 8
N_H, N_L, F, D, K = 16384, 4096, 256, 3, 3
QPC = N_H // N_CORES          # queries per core
NT = QPC // 128               # 128-query tiles per core
EPS = 1e-16

f32 = mybir.dt.float32
u32 = mybir.dt.uint32


def _build_kernel_body(nc, tc, ctx, x_ap, pl_ap, ph_ap, y_ap):
    const = ctx.enter_context(tc.tile_pool(name="const", bufs=1))
    work = ctx.enter_context(tc.tile_pool(name="work", bufs=2))

    ident = const.tile([128, 128], f32)
    masks.make_identity(nc, ident[:])

    aug_pl = const.tile([4, N_L], f32)     # rows 0-2: 2*pl^T, row 3: -|pl|^2
    aug_ph = const.tile([4, QPC], f32)     # rows 0-2: ph^T,   row 3: 1
    phc = const.tile([128, NT, D], f32)    # ph coords, query-tile layout

    # ---- coarse side: transpose pos_l into [3, N_L], build aug_pl ----
    with tc.tile_pool(name="psA", bufs=1, space="PSUM") as psA:
        plT = psA.tile([3, N_L], f32)
        for a in range(N_L // 128):
            t_in = work.tile([128, D], f32, tag="pl_in")
            nc.sync.dma_start(t_in[:], pl_ap[a * 128:(a + 1) * 128, :])
            nc.tensor.transpose(plT[:, a * 128:(a + 1) * 128], t_in[:], ident[:])
        nc.vector.tensor_scalar_mul(aug_pl[0:3, :], plT[:], 2.0)

    sq = const.tile([3, N_L], f32)
    nc.vector.tensor_tensor(sq[:], aug_pl[0:3, :], aug_pl[0:3, :],
                            op=mybir.AluOpType.mult)     # (2*pl)^2

    with tc.tile_pool(name="psB", bufs=1, space="PSUM") as psB:
        nrm = psB.tile([1, N_L], f32)
        ones3 = const.tile([3, 1], f32)
        nc.vector.memset(ones3[:], -0.25)                # sum((2pl)^2) * -1/4
        for b in range(N_L // 512):
            nc.tensor.matmul(nrm[:, b * 512:(b + 1) * 512], ones3[:],
                             sq[:, b * 512:(b + 1) * 512])
        nrm_sb = const.tile([1, N_L], f32)
        nc.scalar.copy(nrm_sb[:], nrm[:])                # PSUM -> SBUF p0
        nc.sync.dma_start(aug_pl[3:4, :], nrm_sb[:])     # -> partition 3

    # ---- query side: transpose pos_h into [3, QPC] ----
    with tc.tile_pool(name="psC", bufs=1, space="PSUM") as psC:
        phT = psC.tile([3, QPC], f32)
        for t in range(NT):
            nc.sync.dma_start(phc[:, t, :], ph_ap[t * 128:(t + 1) * 128, :])
            nc.tensor.transpose(phT[:, t * 128:(t + 1) * 128], phc[:, t, :],
                                ident[:])
        nc.vector.tensor_scalar_mul(aug_ph[0:3, :], phT[:], 1.0)
    ones_row = const.tile([1, QPC], f32)
    nc.vector.memset(ones_row[:], 1.0)
    nc.sync.dma_start(aug_ph[3:4, :], ones_row[:])

    # ---- main loop over 128-query tiles ----
    ps = ctx.enter_context(tc.tile_pool(name="ps", bufs=1, space="PSUM"))
    sm = ctx.enter_context(tc.tile_pool(name="sm", bufs=3))
    fp = ctx.enter_context(tc.tile_pool(name="fp", bufs=2))
    op = ctx.enter_context(tc.tile_pool(name="op", bufs=3))

    for t in range(NT):
        scores = ps.tile([128, N_L], f32, tag="scores")
        for b in range(N_L // 512):
            nc.tensor.matmul(scores[:, b * 512:(b + 1) * 512],
                             aug_ph[:, t * 128:(t + 1) * 128],
                             aug_pl[:, b * 512:(b + 1) * 512])

        t8 = sm.tile([128, 8], f32, tag="t8")
        i8 = sm.tile([128, 8], u32, tag="i8")
        nc.vector.max(t8[:], scores[:])
        nc.vector.max_index(i8[:], t8[:], scores[:])

        feats = fp.tile([128, K, F], f32, tag="feats")
        for k in range(K):
            nc.gpsimd.indirect_dma_start(
                feats[:, k, :], None, x_ap[:, :],
                IndirectOffsetOnAxis(ap=i8[:, k:k + 1], axis=0))

        # exact d2 recompute: gather pos_l[idx], diff^2 against query coords
        plsel = sm.tile([128, K, D], f32, tag="plsel")
        for k in range(K):
            nc.gpsimd.indirect_dma_start(
                plsel[:, k, :], None, pl_ap[:, :],
                IndirectOffsetOnAxis(ap=i8[:, k:k + 1], axis=0))
        qb = phc[:, t, :].unsqueeze(1).broadcast_to((128, K, D))
        dif = sm.tile([128, K, D], f32, tag="dif")
        nc.vector.tensor_tensor(dif[:], plsel[:], qb,
                                op=mybir.AluOpType.subtract)
        nc.vector.tensor_tensor(dif[:], dif[:], dif[:],
                                op=mybir.AluOpType.mult)
        w = sm.tile([128, K], f32, tag="w")
        nc.vector.tensor_reduce(w[:], dif[:], axis=mybir.AxisListType.X,
                                op=mybir.AluOpType.add)
        nc.vector.tensor_scalar(w[:], w[:], EPS, None,
                                op0=mybir.AluOpType.max)
        nc.vector.reciprocal(w[:], w[:])
        wsum = sm.tile([128, 1], f32, tag="wsum")
        nc.vector.tensor_reduce(wsum[:], w[:], axis=mybir.AxisListType.X,
                                op=mybir.AluOpType.add)
        nc.vector.reciprocal(wsum[:], wsum[:])

        acc = op.tile([128, F], f32, tag="acc")
        tmp = op.tile([128, F], f32, tag="tmp")
        nc.vector.tensor_scalar_mul(acc[:], feats[:, 0, :], w[:, 0:1])
        nc.scalar.activation(tmp[:], feats[:, 1, :],
                             mybir.ActivationFunctionType.Copy,
                             scale=w[:, 1:2])
        nc.vector.tensor_tensor(acc[:], acc[:], tmp[:],
                                op=mybir.AluOpType.add)
        nc.scalar.activation(tmp[:], feats[:, 2, :],
                             mybir.ActivationFunctionType.Copy,
                             scale=w[:, 2:3])
        nc.vector.tensor_tensor(acc[:], acc[:], tmp[:],
                                op=mybir.AluOpType.add)
        nc.vector.tensor_scalar_mul(acc[:], acc[:], wsum[:])
        nc.sync.dma_start(y_ap[t * 128:(t + 1) * 128, :], acc[:])


_CACHE = {}


def _get_program():
    if "nc" in _CACHE:
        return _CACHE["nc"]
    nc = bacc.Bacc("TRN2", target_bir_lowering=False, debug=False,
                   num_devices=N_CORES)
    x_t = nc.dram_tensor("x", (N_L, F), f32, kind="ExternalInput")
    pl_t = nc.dram_tensor("pos_l", (N_L, D), f32, kind="ExternalInput")
    ph_t = nc.dram_tensor("pos_h", (QPC, D), f32, kind="ExternalInput")
    y_t = nc.dram_tensor("y", (QPC, F), f32, kind="ExternalOutput")
    with tile.TileContext(nc) as tc, ExitStack() as ctx:
        _build_kernel_body(nc, tc, ctx, x_t.ap(), pl_t.ap(), ph_t.ap(),
                           y_t.ap())
    nc.compile()
    _CACHE["nc"] = nc
    return nc


def kernel(x: np.ndarray, pos_l: np.ndarray, pos_h: np.ndarray) -> np.ndarray:
    x = np.ascontiguousarray(x, dtype=np.float32)
    pos_l = np.ascontiguousarray(pos_l, dtype=np.float32)
    pos_h = np.ascontiguousarray(pos_h, dtype=np.float32)
    nc = _get_program()
    in_maps = [
        {"x": x, "pos_l": pos_l,
         "pos_h": pos_h[c * QPC:(c + 1) * QPC]}
        for c in range(N_CORES)
    ]
    res = bass_utils.run_bass_kernel_spmd(
        nc, in_maps, core_ids=list(range(N_CORES)), trace=False)
    out = np.concatenate([r["y"] for r in res.results], axis=0)
    _CACHE["last_results"] = res
    return out


def benchmark(x, pos_l, pos_h, iters=50, warmup=5):
    """Steady-state per-call device time via pipelined PJRT dispatch.

    Builds the same shard_map(jit) as run_bass_via_pjrt but without
    donation (the kernel writes every output element), keeps inputs
    device-resident, and times `iters` back-to-back dispatches.
    """
    import time

    import jax
    from jax.sharding import Mesh, PartitionSpec
    from jax.experimental.shard_map import shard_map
    from concourse import bass2jax
    from concourse.bass2jax import (_bass_exec_p, install_neuronx_cc_hook,
                                    partition_id_tensor)

    nc = _get_program()
    install_neuronx_cc_hook()

    part_name = (nc.partition_id_tensor.name
                 if nc.partition_id_tensor else None)
    in_names, out_names, out_avals, zero_outs = [], [], [], []
    import concourse.mybir as mb
    for alloc in nc.m.functions[0].allocations:
        if not isinstance(alloc, mb.MemoryLocationSet):
            continue
        name = alloc.memorylocations[0].name
        if alloc.kind == "ExternalInput":
            if name != part_name:
                in_names.append(name)
        elif alloc.kind == "ExternalOutput":
            out_names.append(name)
            shape = tuple(alloc.tensor_shape)
            dtype = mb.dt.np(alloc.dtype)
            out_avals.append(jax.core.ShapedArray(shape, dtype))
            zero_outs.append(np.zeros(shape, dtype))
    n_params = len(in_names)
    all_in_names = in_names + out_names
    if part_name is not None:
        all_in_names = all_in_names + [part_name]

    def _body(*args):
        operands = list(args)
        if part_name is not None:
            operands.append(partition_id_tensor())
        outs = _bass_exec_p.bind(
            *operands,
            out_avals=tuple(out_avals),
            in_names=tuple(all_in_names),
            out_names=tuple(out_names),
            lowering_input_output_aliases=(),
            sim_require_finite=True,
            sim_require_nnan=True,
            nc=nc,
        )
        return tuple(outs)

    devices = jax.devices()[:N_CORES]
    mesh = Mesh(np.asarray(devices), ("core",))
    nspec = n_params + len(out_names)
    sharded = jax.jit(
        shard_map(_body, mesh=mesh,
                  in_specs=(PartitionSpec("core"),) * nspec,
                  out_specs=(PartitionSpec("core"),) * len(out_names),
                  check_rep=False),
        keep_unused=True)

    per_core = [
        {"x": x, "pos_l": pos_l, "pos_h": pos_h[c * QPC:(c + 1) * QPC]}
        for c in range(N_CORES)
    ]
    concat_in = [
        np.concatenate([per_core[c][nm] for c in range(N_CORES)], axis=0)
        for nm in in_names
    ]
    concat_zero = [
        np.zeros((N_CORES * z.shape[0], *z.shape[1:]), z.dtype)
        for z in zero_outs
    ]
    args = [jax.device_put(a) for a in concat_in + concat_zero]

    for _ in range(warmup):
        outs = sharded(*args)
    jax.block_until_ready(outs)
    t0 = time.perf_counter()
    for _ in range(iters):
        outs = sharded(*args)
    jax.block_until_ready(outs)
    t1 = time.perf_counter()
    per_call_ns = (t1 - t0) / iters * 1e9
    y = np.asarray(outs[out_names.index("y")])
    y = y.reshape(N_CORES, QPC, F).reshape(N_H, F)
    return per_call_ns, y


if __name__ == "__main__":
    # quick smoke: build the program and print instruction count
    nc = _get_program()
    n = sum(len(b.instructions) for b in nc.m.functions[0].blocks)
    print(f"program built: {n} instructions")
